# revision 20
# baseline (speedup 1.0000x reference)
"""Trainium2 Bass kernel for nn_ExemplarSoftmaxLoss (data-parallel over 8 cores).

v4 strategy:
  - Host-side: rows of each core's shard are PERMUTED (all reductions are
    permutation-invariant): distance rows + xout thirds 0/1 sorted by
    labels_anchor, xout third 2 sorted by labels_neg.  Sorted rows make
    each 128-row block's labels fall in a narrow window, so the label-logit
    extraction only scans a static W-column window.  Window bases/width are
    computed from the data before compile (kernel builds lazily).
  - All bulk inputs are uploaded as bf16 (the 2e-2 rel-err budget makes
    mixed precision the right kernel design): halves HBM traffic to
    ~22.8 MB/core (~64 us of DMA) and enables the DVE 2x bf16 mode for
    the distance diffs.
  - The distance phase runs in a TRANSPOSED layout: anchor/pos/neg are
    uploaded as [D, BS] and exemplar rows are fetched with
    dma_gather(transpose=True), so diff tiles are [d-partition, row-free].
    Row sum-of-squares then runs on the otherwise-idle TensorEngine as
    diagonal matmuls df.T @ df (PSUM-accumulated over the 4 d-chunks);
    the diagonal is pulled out with a 128-wide is_equal STT.  This removes
    all 96 square ops (~60 us of Scalar+DVE) from the critical engines.
  - ScalarE runs the exp stream only; DVE does extraction + diffs + diag.
  - Host: float64 reduction of the 8x[128,4] partials -> 4 scalar losses.
"""

import os
import sys

import numpy as np
import ml_dtypes

for _p in ("/opt/trn_rl_repo",):
    if _p not in sys.path and os.path.isdir(_p):
        sys.path.insert(0, _p)

import concourse.bass as bass
import concourse.tile as tile
from concourse import bacc, mybir
from concourse._compat import with_exitstack
from concourse.bass_utils import run_bass_kernel_spmd

try:
    import antenv.axon_hooks  # noqa: F401
except ImportError:
    import types as _types

    _m = _types.ModuleType("antenv.axon_hooks")
    _m.get_axon_ntff_profile_hook = lambda: None
    _m.set_axon_ntff_profile_hook = lambda h: None
    sys.modules["antenv.axon_hooks"] = _m

# Problem constants (hardcoded per the harness contract).
B, D, C = 16384, 512, 1000
NCORES = 8
BS = B // NCORES  # 2048 batch rows per core
RS = 3 * BS  # 6144 softmax rows per core
P = 128
NB = BS // P  # 16 row-blocks in the distance phase
NR = RS // P  # 48 row-blocks in the softmax phase
NG = 4  # groups of 4 row-blocks in the distance phase
DC = D // P  # 4 d-chunks in the transposed layout
EPS = 1e-6
MARGIN2 = 0.2
LAMBDA = 1.0

# xout DMA tiles: n blocks each; 2-block head tiles give the exp stream an
# early start; the rest are 4-block (1MB bf16) tiles.
TILE_SHAPES = [2, 2, 4, 4, 4]
TILES = [
    (t, sum(TILE_SHAPES[:i]), TILE_SHAPES[i])
    for t in range(3)
    for i in range(len(TILE_SHAPES))
]
NXT = len(TILES)  # 15

f32 = mybir.dt.float32
bf16 = mybir.dt.bfloat16
i16 = mybir.dt.int16
Alu = mybir.AluOpType
Act = mybir.ActivationFunctionType
AX = mybir.AxisListType

LAST_RESULTS = None  # BassKernelResults of the most recent run (for test.py)


@with_exitstack
def _emit(ctx, tc, outs, ins, bases, W):
    nc = tc.nc
    xo = ins["xout"]  # [RS, C] bf16 (3 thirds, host-permuted)
    aa = ins["anc"]  # [D, BS] bf16 transposed (cols sorted by la)
    pp = ins["pos"]  # [D, BS] bf16
    ng = ins["neg"]  # [D, BS] bf16
    ex = ins["exem"]  # [C, D]  bf16 exemplar table
    ia = ins["idxa"]  # [128, 128] i16 wrapped gather idx (= sorted la)
    in_ = ins["idxn"]  # [128, 128] i16 wrapped gather idx (= ln[perm_a])
    lsh = ins["labsh"]  # [P, NR] f32 label - window_base per block
    pd = outs["partials"]  # [P, 4] f32

    sing = ctx.enter_context(tc.tile_pool(name="sing", bufs=1))
    xpool = ctx.enter_context(tc.tile_pool(name="xp", bufs=6))
    ejp = ctx.enter_context(tc.tile_pool(name="ejp", bufs=2, space="PSUM"))
    mmp = ctx.enter_context(tc.tile_pool(name="mmp", bufs=4, space="PSUM"))
    ljp = ctx.enter_context(tc.tile_pool(name="ljp", bufs=3))
    dgp = ctx.enter_context(tc.tile_pool(name="dgp", bufs=4))
    dfp = ctx.enter_context(tc.tile_pool(name="dfp", bufs=4))

    sums = sing.tile([P, NR], f32)  # per-row sum(exp(x))
    lbl = sing.tile([P, NR], f32)  # label logits per block
    d2a = sing.tile([P, NB * 3], f32)  # sq dists: dr1,dn1,dr2
    d2v = sing.tile([P, NB * 3], f32)  # sq dists: dn2,tp,tn
    iota_w = sing.tile([P, W], f32)
    pidx = sing.tile([P, 1], f32)  # value = partition index
    lsh_t = sing.tile([P, NR], f32)
    ia_t = sing.tile([128, 128], i16)
    in_t = sing.tile([128, 128], i16)
    # transposed distance operands: tile[p, c, r] = X[r, c*128+p]
    at = sing.tile([P, DC, BS], bf16)
    pt = sing.tile([P, DC, BS], bf16)
    nt = sing.tile([P, DC, BS], bf16)
    # gather output must have contiguous free dims per call -> group-major
    exa = sing.tile([P, NG, DC, 512], bf16)
    exn = sing.tile([P, NG, DC, 512], bf16)

    # small loads go on the scalar HWDGE queue: the pool-dynamic SDMA queue
    # is starved while the sync queue streams, and the sync queue's xout
    # tiles would delay these past 20us -- either way delaying the first
    # gather (which waits on the idx loads).
    nc.scalar.dma_start(out=lsh_t[:], in_=lsh[:])
    nc.scalar.dma_start(out=ia_t[:], in_=ia[:])
    nc.scalar.dma_start(out=in_t[:], in_=in_[:])
    nc.gpsimd.iota(
        iota_w[:],
        pattern=[[1, W]],
        base=0,
        channel_multiplier=0,
        allow_small_or_imprecise_dtypes=True,
    )
    nc.gpsimd.iota(
        pidx[:],
        pattern=[[1, 1]],
        base=0,
        channel_multiplier=1,
        allow_small_or_imprecise_dtypes=True,
    )
    for g in range(NG):
        for dst, idx in ((exa, ia_t), (exn, in_t)):
            nc.gpsimd.dma_gather(
                dst[:, g],
                ex[:],
                idx[:, 32 * g : 32 * g + 32],
                512,
                512,
                D,
                transpose=True,
            )

    xo3 = xo.rearrange("(t r) c -> t r c", t=3)
    xt_tiles = {}

    def emit_xload(s):
        t, j0, nb = TILES[s]
        xt = xpool.tile([P, nb, C], bf16, tag="xt", name=f"xt{s}")
        nc.sync.dma_start(
            out=xt[:],
            in_=xo3[t, j0 * P : (j0 + nb) * P, :].rearrange(
                "(t p) c -> p t c", p=P
            ),
        )
        xt_tiles[s] = xt

    def emit_xcompute(s):
        xt = xt_tiles.pop(s)
        t, j0, nb = TILES[s]
        for b in range(nb):
            j = j0 + b  # block index within the third
            col = 16 * t + j
            ej = ejp.tile([P, C], f32, tag="ej")
            nc.scalar.activation(
                out=ej[:],
                in_=xt[:, b, :],
                func=Act.Exp,
                accum_out=sums[:, col : col + 1],
            )
            base = bases[j]
            lj = ljp.tile([P, W], f32, tag="lj")
            nc.vector.scalar_tensor_tensor(
                out=lj[:],
                in0=iota_w[:],
                scalar=lsh_t[:, col : col + 1],
                in1=xt[:, b, base : base + W],
                op0=Alu.is_equal,
                op1=Alu.mult,
                accum_out=lbl[:, col : col + 1],
            )

    def emit_apn_loads(g):
        # scalar-engine HWDGE ring: separate FIFO from the sync queue, so
        # these don't delay the xout tile stream
        r0, r1 = 512 * g, 512 * (g + 1)
        for dst, src in ((at, aa), (pt, pp), (nt, ng)):
            nc.scalar.dma_start(
                out=dst[:, :, r0:r1],
                in_=src[:, r0:r1].rearrange("(c p) r -> p c r", p=P),
            )

    def emit_diag(df, rcl, d2t, ci, g):
        # mm = df_chunk.T @ df_chunk accumulated over the 4 d-chunks;
        # diag(mm)[p] = sum_d df[d, blk*128+p]^2 = d^2 of row blk*128+p
        blk = 4 * g + rcl
        rsl = slice(128 * rcl, 128 * (rcl + 1))
        mm = mmp.tile([P, P], f32, tag="mm")
        for dc in range(DC):
            nc.tensor.matmul(
                out=mm[:],
                lhsT=df[:, dc, rsl],
                rhs=df[:, dc, rsl],
                start=(dc == 0),
                stop=(dc == DC - 1),
            )
        dg = dgp.tile([P, P], f32, tag="dg")
        nc.vector.scalar_tensor_tensor(
            out=dg[:],
            in0=iota_w[:, 0:P],
            scalar=pidx[:],
            in1=mm[:],
            op0=Alu.is_equal,
            op1=Alu.mult,
            accum_out=d2t[:, blk * 3 + ci : blk * 3 + ci + 1],
        )

    def emit_group(g):
        rsl = slice(512 * g, 512 * (g + 1))
        pairs = (
            (at, exa, d2a, 0),  # d_ref1
            (nt, exa, d2a, 1),  # d_neg1
            (at, exn, d2a, 2),  # d_ref2
            (nt, exn, d2v, 0),  # d_neg2
            (at, pt, d2v, 1),  # tp
            (at, nt, d2v, 2),  # tn
        )
        dfs = []
        # software-pipeline: diffs lead their diag extraction by one pair so
        # the DVE never waits on the PE matmuls.  The tp/tn diffs don't need
        # gathered data, so they run on the (otherwise idle) Pool engine.
        for pi, (xs, ys, d2t, ci) in enumerate(pairs):
            in0 = xs[:, :, rsl]
            in1 = ys[:, g] if (ys is exa or ys is exn) else ys[:, :, rsl]
            eng = nc.gpsimd if pi >= 4 else nc.vector
            df = dfp.tile([P, DC, 512], bf16, tag="df")
            eng.tensor_tensor(
                out=df[:], in0=in0, in1=in1, op=Alu.subtract
            )
            dfs.append((df, d2t, ci))
            if pi >= 1:
                df0, d2t0, ci0 = dfs[pi - 1]
                for rcl in range(4):
                    emit_diag(df0, rcl, d2t0, ci0, g)
        df0, d2t0, ci0 = dfs[-1]
        for rcl in range(4):
            emit_diag(df0, rcl, d2t0, ci0, g)

    # ---- main schedule ----
    emit_xload(0)
    emit_xload(1)
    emit_apn_loads(0)
    emit_xload(2)
    emit_apn_loads(1)

    for s in range(NXT):
        if s + 3 < NXT:
            emit_xload(s + 3)
        if s == 2:
            emit_apn_loads(2)
        if s == 4:
            emit_apn_loads(3)
        emit_xcompute(s)
        if s in (4, 6, 8, 10):
            emit_group((s - 4) // 2)

    # ---- tail ----
    part = sing.tile([P, 4], f32)
    dda = sing.tile([P, NB * 3], f32)
    ddv = sing.tile([P, NB * 3], f32)
    nc.scalar.activation(out=dda[:], in_=d2a[:], func=Act.Sqrt)
    nc.scalar.activation(out=ddv[:], in_=d2v[:], func=Act.Sqrt)
    logs = sing.tile([P, NR], f32)
    nc.scalar.activation(out=logs[:], in_=sums[:], func=Act.Ln)
    nc.vector.reduce_sum(out=part[:, 0:1], in_=logs[:], axis=AX.X)
    nc.vector.reduce_sum(out=part[:, 1:2], in_=lbl[:], axis=AX.X)

    dA = dda[:].rearrange("p (b k) -> p b k", k=3)
    dV = ddv[:].rearrange("p (b k) -> p b k", k=3)

    x1 = sing.tile([P, NB], f32)
    m1 = sing.tile([P, NB], f32)
    c1 = sing.tile([P, NB], f32)
    x2 = sing.tile([P, NB], f32)
    c2 = sing.tile([P, NB], f32)
    x3 = sing.tile([P, NB], f32)
    t3 = sing.tile([P, NB], f32)
    ca = sing.tile([P, 1], f32)
    cb = sing.tile([P, 1], f32)

    # c1 = (dr1 - dn1 > 0) ? (dr1 - dn1 + MARGIN2) : 0
    nc.vector.tensor_tensor(out=x1[:], in0=dA[:, :, 0], in1=dA[:, :, 1], op=Alu.subtract)
    nc.vector.tensor_scalar(
        out=m1[:], in0=x1[:], scalar1=0.0, scalar2=None, op0=Alu.is_gt
    )
    nc.vector.scalar_tensor_tensor(
        out=c1[:], in0=x1[:], scalar=MARGIN2, in1=m1[:],
        op0=Alu.add, op1=Alu.mult, accum_out=ca[:],
    )
    # c2 = relu(dn2 - dr2)
    nc.vector.tensor_tensor(out=x2[:], in0=dV[:, :, 0], in1=dA[:, :, 2], op=Alu.subtract)
    nc.vector.tensor_scalar(
        out=c2[:], in0=x2[:], scalar1=0.0, scalar2=None,
        op0=Alu.max, op1=Alu.add, accum_out=cb[:],
    )
    # t = relu(tp - tn)
    nc.vector.tensor_tensor(out=x3[:], in0=dV[:, :, 1], in1=dV[:, :, 2], op=Alu.subtract)
    nc.vector.tensor_scalar(
        out=t3[:], in0=x3[:], scalar1=0.0, scalar2=None,
        op0=Alu.max, op1=Alu.add, accum_out=part[:, 3:4],
    )
    nc.vector.tensor_tensor(out=part[:, 2:3], in0=ca[:], in1=cb[:], op=Alu.add)
    nc.sync.dma_start(out=pd[:], in_=part[:])


_COMPILED = {}


def _build(bases, W):
    key = (tuple(bases), W)
    if key in _COMPILED:
        return _COMPILED[key]
    nc = bacc.Bacc(
        "TRN2",
        target_bir_lowering=False,
        debug=False,
        enable_asserts=False,
        num_devices=NCORES,
    )
    ins = {
        "xout": nc.dram_tensor("xout", [RS, C], bf16, kind="ExternalInput").ap(),
        "anc": nc.dram_tensor("anc", [D, BS], bf16, kind="ExternalInput").ap(),
        "pos": nc.dram_tensor("pos", [D, BS], bf16, kind="ExternalInput").ap(),
        "neg": nc.dram_tensor("neg", [D, BS], bf16, kind="ExternalInput").ap(),
        "exem": nc.dram_tensor("exem", [C, D], bf16, kind="ExternalInput").ap(),
        "idxa": nc.dram_tensor("idxa", [128, 128], i16, kind="ExternalInput").ap(),
        "idxn": nc.dram_tensor("idxn", [128, 128], i16, kind="ExternalInput").ap(),
        "labsh": nc.dram_tensor("labsh", [P, NR], f32, kind="ExternalInput").ap(),
    }
    outs = {
        "partials": nc.dram_tensor("partials", [P, 4], f32, kind="ExternalOutput").ap()
    }
    with tile.TileContext(nc) as tc:
        _emit(tc, outs, ins, bases, W)
    nc.compile()
    _COMPILED[key] = nc
    return nc


def _wrap_idx(v):
    # dma_gather index layout: idx i at [i % 16, i // 16], replicated to
    # each 16-partition group (one per Q7 core).
    w = np.asarray(v, np.int16).reshape(128, 16).T  # [16, 128]
    return np.ascontiguousarray(np.tile(w, (8, 1)))  # [128, 128]


def _bf16(a):
    return np.ascontiguousarray(np.asarray(a, np.float32).astype(ml_dtypes.bfloat16))


def _prep(anchor, positive, negative, outputs, labels_anchor, labels_neg, exemplars):
    anchor = np.asarray(anchor, np.float32)
    positive = np.asarray(positive, np.float32)
    negative = np.asarray(negative, np.float32)
    outputs = np.asarray(outputs, np.float32)
    ex16 = _bf16(exemplars)
    la_all = np.asarray(labels_anchor).astype(np.int64)
    ln_all = np.asarray(labels_neg).astype(np.int64)

    cores = []
    lo = np.full(NB, C, np.int64)
    hi = np.full(NB, -1, np.int64)
    for k in range(NCORES):
        sl = slice(k * BS, (k + 1) * BS)
        la, ln = la_all[sl], ln_all[sl]
        pa = np.argsort(la, kind="stable")
        pn = np.argsort(ln, kind="stable")
        la_s, ln_s = la[pa], ln[pn]
        for v in (la_s, ln_s):
            vb = v.reshape(NB, P)
            np.minimum(lo, vb.min(axis=1), out=lo)
            np.maximum(hi, vb.max(axis=1), out=hi)
        cores.append((k, sl, pa, pn, la_s, ln_s))

    span = int((hi - lo).max()) + 1
    W = max(128, -(-span // 32) * 32)
    assert W <= C, f"label window infeasible: span {span}"
    bases = np.minimum(np.minimum(lo, C - W), hi - W + 1)
    bases = np.maximum(bases, 0).astype(np.int64)
    assert ((bases <= lo) & (bases + W > hi)).all()

    maps = []
    for k, sl, pa, pn, la_s, ln_s in cores:
        ln_pa = ln_all[sl][pa]
        xo = np.concatenate(
            [
                outputs[k * BS : (k + 1) * BS][pa],
                outputs[B + k * BS : B + (k + 1) * BS][pa],
                outputs[2 * B + k * BS : 2 * B + (k + 1) * BS][pn],
            ],
            axis=0,
        )
        labsh = np.empty((P, NR), np.float32)
        for t, v in enumerate((la_s, la_s, ln_s)):
            labsh[:, 16 * t : 16 * t + 16] = (
                (v.reshape(NB, P) - bases[:, None]).T.astype(np.float32)
            )
        maps.append(
            {
                "xout": _bf16(xo),
                "anc": _bf16(anchor[sl][pa].T),
                "pos": _bf16(positive[sl][pa].T),
                "neg": _bf16(negative[sl][pa].T),
                "exem": ex16,
                "idxa": _wrap_idx(la_s),
                "idxn": _wrap_idx(ln_pa),
                "labsh": np.ascontiguousarray(labsh),
            }
        )
    return maps, tuple(int(b) for b in bases), W


def _combine(results):
    S = np.zeros(4, dtype=np.float64)
    for r in results:
        S += r["partials"].astype(np.float64).sum(axis=0)
    loss_softmax = (S[0] - S[1]) / (3 * B)
    loss_center = S[2]
    loss_triplet = S[3]
    loss_total = loss_softmax + 0.01 * loss_center + LAMBDA * loss_triplet
    return (
        np.float32(loss_total),
        np.float32(loss_triplet),
        np.float32(loss_softmax),
        np.float32(loss_center),
    )


def kernel(anchor, positive, negative, outputs, labels_anchor, labels_neg, exemplars):
    global LAST_RESULTS
    maps, bases, W = _prep(
        anchor, positive, negative, outputs, labels_anchor, labels_neg, exemplars
    )
    nc = _build(bases, W)
    res = run_bass_kernel_spmd(nc, maps, core_ids=list(range(NCORES)))
    LAST_RESULTS = res
    return _combine(res.results)


# revision 22
# speedup vs baseline: 1.3955x; 1.3955x over previous
"""Trainium2 Bass kernel for nn_ExemplarSoftmaxLoss (data-parallel over 8 cores).

v4 strategy:
  - Host-side: rows of each core's shard are PERMUTED (all reductions are
    permutation-invariant): distance rows + xout thirds 0/1 sorted by
    labels_anchor, xout third 2 sorted by labels_neg.  Sorted rows make
    each 128-row block's labels fall in a narrow window, so the label-logit
    extraction only scans a static W-column window.  Window bases/width are
    computed from the data before compile (kernel builds lazily).
  - All bulk inputs are uploaded as bf16 (the 2e-2 rel-err budget makes
    mixed precision the right kernel design): halves HBM traffic to
    ~22.8 MB/core (~64 us of DMA) and enables the DVE 2x bf16 mode for
    the distance diffs.
  - The distance phase runs in a TRANSPOSED layout: anchor/pos/neg are
    uploaded as [D, BS] and exemplar rows are fetched with
    dma_gather(transpose=True), so diff tiles are [d-partition, row-free].
    Row sum-of-squares then runs on the otherwise-idle TensorEngine as
    diagonal matmuls df.T @ df (PSUM-accumulated over the 4 d-chunks);
    the diagonal is pulled out with a 128-wide is_equal STT.  This removes
    all 96 square ops (~60 us of Scalar+DVE) from the critical engines.
  - ScalarE runs the exp stream only; DVE does extraction + diffs + diag.
  - Host: float64 reduction of the 8x[128,4] partials -> 4 scalar losses.
"""

import os
import sys

import numpy as np
import ml_dtypes

for _p in ("/opt/trn_rl_repo",):
    if _p not in sys.path and os.path.isdir(_p):
        sys.path.insert(0, _p)

import concourse.bass as bass
import concourse.tile as tile
from concourse import bacc, mybir
from concourse._compat import with_exitstack
from concourse.bass_utils import run_bass_kernel_spmd

try:
    import antenv.axon_hooks  # noqa: F401
except ImportError:
    import types as _types

    _m = _types.ModuleType("antenv.axon_hooks")
    _m.get_axon_ntff_profile_hook = lambda: None
    _m.set_axon_ntff_profile_hook = lambda h: None
    sys.modules["antenv.axon_hooks"] = _m

# Problem constants (hardcoded per the harness contract).
B, D, C = 16384, 512, 1000
NCORES = 8
BS = B // NCORES  # 2048 batch rows per core
RS = 3 * BS  # 6144 softmax rows per core
P = 128
NB = BS // P  # 16 row-blocks in the distance phase
NR = RS // P  # 48 row-blocks in the softmax phase
NG = 4  # groups of 4 row-blocks in the distance phase
DC = D // P  # 4 d-chunks in the transposed layout
EPS = 1e-6
MARGIN2 = 0.2
LAMBDA = 1.0

# xout DMA tiles: n blocks each; 2-block head tiles give the exp stream an
# early start; the rest are 4-block (1MB bf16) tiles.
TILE_SHAPES = [2, 2, 4, 4, 4]
TILES = [
    (t, sum(TILE_SHAPES[:i]), TILE_SHAPES[i])
    for t in range(3)
    for i in range(len(TILE_SHAPES))
]
NXT = len(TILES)  # 15

f32 = mybir.dt.float32
bf16 = mybir.dt.bfloat16
i16 = mybir.dt.int16
Alu = mybir.AluOpType
Act = mybir.ActivationFunctionType
AX = mybir.AxisListType

LAST_RESULTS = None  # BassKernelResults of the most recent run (for test.py)


@with_exitstack
def _emit(ctx, tc, outs, ins, bases, W):
    nc = tc.nc
    xo = ins["xout"]  # [RS, C] bf16 (3 thirds, host-permuted)
    aa = ins["anc"]  # [D, BS] bf16 transposed (cols sorted by la)
    pp = ins["pos"]  # [D, BS] bf16
    ng = ins["neg"]  # [D, BS] bf16
    ex = ins["exem"]  # [C, D]  bf16 exemplar table
    ia = ins["idxa"]  # [128, 128] i16 wrapped gather idx (= sorted la)
    in_ = ins["idxn"]  # [128, 128] i16 wrapped gather idx (= ln[perm_a])
    lsh = ins["labsh"]  # [P, NR] f32 label - window_base per block
    pd = outs["partials"]  # [P, 4] f32

    sing = ctx.enter_context(tc.tile_pool(name="sing", bufs=1))
    xpool = ctx.enter_context(tc.tile_pool(name="xp", bufs=6))
    ejp = ctx.enter_context(tc.tile_pool(name="ejp", bufs=2, space="PSUM"))
    mmp = ctx.enter_context(tc.tile_pool(name="mmp", bufs=4, space="PSUM"))
    ljp = ctx.enter_context(tc.tile_pool(name="ljp", bufs=3))
    dgp = ctx.enter_context(tc.tile_pool(name="dgp", bufs=4))
    dfp = ctx.enter_context(tc.tile_pool(name="dfp", bufs=4))

    sums = sing.tile([P, NR], f32)  # per-row sum(exp(x))
    lbl = sing.tile([P, NR], f32)  # label logits per block
    d2a = sing.tile([P, NB * 3], f32)  # sq dists: dr1,dn1,dr2
    d2v = sing.tile([P, NB * 3], f32)  # sq dists: dn2,tp,tn
    iota_w = sing.tile([P, W], f32)
    pidx = sing.tile([P, 1], f32)  # value = partition index
    lsh_t = sing.tile([P, 128], f32)
    ia_t = sing.tile([128, 256], i16)
    in_t = sing.tile([128, 256], i16)
    # transposed distance operands: tile[p, c, r] = X[r, c*128+p]
    part = sing.tile([P, 128], f32)  # [:, :4] = loss partials
    at = sing.tile([P, DC, BS], bf16)
    pt = sing.tile([P, DC, BS], bf16)
    nt = sing.tile([P, DC, BS], bf16)
    # gather output must have contiguous free dims per call -> group-major
    exa = sing.tile([P, NG, DC, 512], bf16)
    exn = sing.tile([P, NG, DC, 512], bf16)

    # small loads go on the scalar HWDGE queue: the pool-dynamic SDMA queue
    # is starved while the sync queue streams, and the sync queue's xout
    # tiles would delay these past 20us -- either way delaying the first
    # gather (which waits on the idx loads).
    nc.scalar.dma_start(out=lsh_t[:], in_=lsh[:])
    nc.scalar.dma_start(out=ia_t[:], in_=ia[:])
    nc.scalar.dma_start(out=in_t[:], in_=in_[:])
    nc.gpsimd.iota(
        iota_w[:],
        pattern=[[1, W]],
        base=0,
        channel_multiplier=0,
        allow_small_or_imprecise_dtypes=True,
    )
    nc.gpsimd.iota(
        pidx[:],
        pattern=[[1, 1]],
        base=0,
        channel_multiplier=1,
        allow_small_or_imprecise_dtypes=True,
    )
    for g in range(NG):
        for dst, idx in ((exa, ia_t), (exn, in_t)):
            nc.gpsimd.dma_gather(
                dst[:, g],
                ex[:],
                idx[:, 32 * g : 32 * g + 32],
                512,
                512,
                D,
                transpose=True,
            )

    nc.gpsimd.memset(part[:], 0.0)

    xo3 = xo.rearrange("(t r) c -> t r c", t=3)
    xt_tiles = {}

    def emit_xload(s):
        t, j0, nb = TILES[s]
        xt = xpool.tile([P, nb, C], bf16, tag="xt", name=f"xt{s}")
        nc.sync.dma_start(
            out=xt[:],
            in_=xo3[t, j0 * P : (j0 + nb) * P, :].rearrange(
                "(t p) c -> p t c", p=P
            ),
        )
        xt_tiles[s] = xt

    def emit_xcompute(s):
        xt = xt_tiles.pop(s)
        t, j0, nb = TILES[s]
        for b in range(nb):
            j = j0 + b  # block index within the third
            col = 16 * t + j
            ej = ejp.tile([P, C], f32, tag="ej")
            nc.scalar.activation(
                out=ej[:],
                in_=xt[:, b, :],
                func=Act.Exp,
                accum_out=sums[:, col : col + 1],
            )
            base = bases[j]
            lj = ljp.tile([P, W], f32, tag="lj")
            nc.vector.scalar_tensor_tensor(
                out=lj[:],
                in0=iota_w[:],
                scalar=lsh_t[:, col : col + 1],
                in1=xt[:, b, base : base + W],
                op0=Alu.is_equal,
                op1=Alu.mult,
                accum_out=lbl[:, col : col + 1],
            )

    def emit_apn_loads(g):
        # scalar-engine HWDGE ring: separate FIFO from the sync queue, so
        # these don't delay the xout tile stream
        r0, r1 = 512 * g, 512 * (g + 1)
        for dst, src in ((at, aa), (pt, pp), (nt, ng)):
            nc.scalar.dma_start(
                out=dst[:, :, r0:r1],
                in_=src[:, r0:r1].rearrange("(c p) r -> p c r", p=P),
            )

    def emit_diag(df, rcl, d2t, ci, g):
        # mm = df_chunk.T @ df_chunk accumulated over the 4 d-chunks;
        # diag(mm)[p] = sum_d df[d, blk*128+p]^2 = d^2 of row blk*128+p
        blk = 4 * g + rcl
        rsl = slice(128 * rcl, 128 * (rcl + 1))
        mm = mmp.tile([P, P], f32, tag="mm")
        for dc in range(DC):
            nc.tensor.matmul(
                out=mm[:],
                lhsT=df[:, dc, rsl],
                rhs=df[:, dc, rsl],
                start=(dc == 0),
                stop=(dc == DC - 1),
            )
        dg = dgp.tile([P, P], f32, tag="dg")
        nc.vector.scalar_tensor_tensor(
            out=dg[:],
            in0=iota_w[:, 0:P],
            scalar=pidx[:],
            in1=mm[:],
            op0=Alu.is_equal,
            op1=Alu.mult,
            accum_out=d2t[:, blk * 3 + ci : blk * 3 + ci + 1],
        )

    def emit_group(g):
        rsl = slice(512 * g, 512 * (g + 1))
        pairs = (
            (at, exa, d2a, 0),  # d_ref1
            (nt, exa, d2a, 1),  # d_neg1
            (at, exn, d2a, 2),  # d_ref2
            (nt, exn, d2v, 0),  # d_neg2
            (at, pt, d2v, 1),  # tp
            (at, nt, d2v, 2),  # tn
        )
        dfs = []
        # software-pipeline: diffs lead their diag extraction by one pair so
        # the DVE never waits on the PE matmuls.  (Pool-engine TT was tried
        # here and regressed badly: ~4.5us per op plus SBUF-port contention
        # that halved the DVE 2x diff rate.)
        for pi, (xs, ys, d2t, ci) in enumerate(pairs):
            in0 = xs[:, :, rsl]
            in1 = ys[:, g] if (ys is exa or ys is exn) else ys[:, :, rsl]
            df = dfp.tile([P, DC, 512], bf16, tag="df")
            nc.vector.tensor_tensor(
                out=df[:], in0=in0, in1=in1, op=Alu.subtract
            )
            dfs.append((df, d2t, ci))
            if pi >= 1:
                df0, d2t0, ci0 = dfs[pi - 1]
                for rcl in range(4):
                    emit_diag(df0, rcl, d2t0, ci0, g)
        df0, d2t0, ci0 = dfs[-1]
        for rcl in range(4):
            emit_diag(df0, rcl, d2t0, ci0, g)

    # ---- main schedule ----
    emit_xload(0)
    emit_xload(1)
    emit_apn_loads(0)
    emit_xload(2)
    emit_apn_loads(1)

    for s in range(NXT):
        if s + 3 < NXT:
            emit_xload(s + 3)
        if s == 2:
            emit_apn_loads(2)
        if s == 4:
            emit_apn_loads(3)
        emit_xcompute(s)
        if s in (4, 6, 8, 10):
            emit_group((s - 4) // 2)

    # ---- tail ----
    dda = sing.tile([P, NB * 3], f32)
    ddv = sing.tile([P, NB * 3], f32)
    nc.scalar.activation(out=dda[:], in_=d2a[:], func=Act.Sqrt)
    nc.scalar.activation(out=ddv[:], in_=d2v[:], func=Act.Sqrt)
    logs = sing.tile([P, NR], f32)
    nc.scalar.activation(out=logs[:], in_=sums[:], func=Act.Ln)
    nc.vector.reduce_sum(out=part[:, 0:1], in_=logs[:], axis=AX.X)
    nc.vector.reduce_sum(out=part[:, 1:2], in_=lbl[:], axis=AX.X)

    dA = dda[:].rearrange("p (b k) -> p b k", k=3)
    dV = ddv[:].rearrange("p (b k) -> p b k", k=3)

    x1 = sing.tile([P, NB], f32)
    m1 = sing.tile([P, NB], f32)
    c1 = sing.tile([P, NB], f32)
    x2 = sing.tile([P, NB], f32)
    c2 = sing.tile([P, NB], f32)
    x3 = sing.tile([P, NB], f32)
    t3 = sing.tile([P, NB], f32)
    ca = sing.tile([P, 1], f32)
    cb = sing.tile([P, 1], f32)

    # c1 = (dr1 - dn1 > 0) ? (dr1 - dn1 + MARGIN2) : 0
    nc.vector.tensor_tensor(out=x1[:], in0=dA[:, :, 0], in1=dA[:, :, 1], op=Alu.subtract)
    nc.vector.tensor_scalar(
        out=m1[:], in0=x1[:], scalar1=0.0, scalar2=None, op0=Alu.is_gt
    )
    nc.vector.scalar_tensor_tensor(
        out=c1[:], in0=x1[:], scalar=MARGIN2, in1=m1[:],
        op0=Alu.add, op1=Alu.mult, accum_out=ca[:],
    )
    # c2 = relu(dn2 - dr2)
    nc.vector.tensor_tensor(out=x2[:], in0=dV[:, :, 0], in1=dA[:, :, 2], op=Alu.subtract)
    nc.vector.tensor_scalar(
        out=c2[:], in0=x2[:], scalar1=0.0, scalar2=None,
        op0=Alu.max, op1=Alu.add, accum_out=cb[:],
    )
    # t = relu(tp - tn)
    nc.vector.tensor_tensor(out=x3[:], in0=dV[:, :, 1], in1=dV[:, :, 2], op=Alu.subtract)
    nc.vector.tensor_scalar(
        out=t3[:], in0=x3[:], scalar1=0.0, scalar2=None,
        op0=Alu.max, op1=Alu.add, accum_out=part[:, 3:4],
    )
    nc.vector.tensor_tensor(out=part[:, 2:3], in0=ca[:], in1=cb[:], op=Alu.add)
    nc.sync.dma_start(out=pd[:], in_=part[:])


_COMPILED = {}


def _build(bases, W):
    key = (tuple(bases), W)
    if key in _COMPILED:
        return _COMPILED[key]
    nc = bacc.Bacc(
        "TRN2",
        target_bir_lowering=False,
        debug=False,
        enable_asserts=False,
        num_devices=NCORES,
    )
    ins = {
        "xout": nc.dram_tensor("xout", [RS, C], bf16, kind="ExternalInput").ap(),
        "anc": nc.dram_tensor("anc", [D, BS], bf16, kind="ExternalInput").ap(),
        "pos": nc.dram_tensor("pos", [D, BS], bf16, kind="ExternalInput").ap(),
        "neg": nc.dram_tensor("neg", [D, BS], bf16, kind="ExternalInput").ap(),
        "exem": nc.dram_tensor("exem", [C, D], bf16, kind="ExternalInput").ap(),
        "idxa": nc.dram_tensor("idxa", [128, 256], i16, kind="ExternalInput").ap(),
        "idxn": nc.dram_tensor("idxn", [128, 256], i16, kind="ExternalInput").ap(),
        "labsh": nc.dram_tensor("labsh", [P, 128], f32, kind="ExternalInput").ap(),
    }
    outs = {
        "partials": nc.dram_tensor("partials", [P, 128], f32, kind="ExternalOutput").ap()
    }
    with tile.TileContext(nc) as tc:
        _emit(tc, outs, ins, bases, W)
    nc.compile()
    _COMPILED[key] = nc
    return nc


def _wrap_idx(v):
    # dma_gather index layout: idx i at [i % 16, i // 16], replicated to
    # each 16-partition group (one per Q7 core).  Padded to 512B/partition
    # rows so the upload DMA avoids the sub-512B RMW descriptor penalty.
    w = np.asarray(v, np.int16).reshape(128, 16).T  # [16, 128]
    out = np.zeros((128, 256), np.int16)
    out[:, :128] = np.tile(w, (8, 1))
    return out


def _bf16(a):
    return np.ascontiguousarray(np.asarray(a, np.float32).astype(ml_dtypes.bfloat16))


def _prep(anchor, positive, negative, outputs, labels_anchor, labels_neg, exemplars):
    anchor = np.asarray(anchor, np.float32)
    positive = np.asarray(positive, np.float32)
    negative = np.asarray(negative, np.float32)
    outputs = np.asarray(outputs, np.float32)
    ex16 = _bf16(exemplars)
    la_all = np.asarray(labels_anchor).astype(np.int64)
    ln_all = np.asarray(labels_neg).astype(np.int64)

    cores = []
    lo = np.full(NB, C, np.int64)
    hi = np.full(NB, -1, np.int64)
    for k in range(NCORES):
        sl = slice(k * BS, (k + 1) * BS)
        la, ln = la_all[sl], ln_all[sl]
        pa = np.argsort(la, kind="stable")
        pn = np.argsort(ln, kind="stable")
        la_s, ln_s = la[pa], ln[pn]
        for v in (la_s, ln_s):
            vb = v.reshape(NB, P)
            np.minimum(lo, vb.min(axis=1), out=lo)
            np.maximum(hi, vb.max(axis=1), out=hi)
        cores.append((k, sl, pa, pn, la_s, ln_s))

    span = int((hi - lo).max()) + 1
    W = max(128, -(-span // 32) * 32)
    assert W <= C, f"label window infeasible: span {span}"
    bases = np.minimum(np.minimum(lo, C - W), hi - W + 1)
    bases = np.maximum(bases, 0).astype(np.int64)
    assert ((bases <= lo) & (bases + W > hi)).all()

    maps = []
    for k, sl, pa, pn, la_s, ln_s in cores:
        ln_pa = ln_all[sl][pa]
        xo = np.concatenate(
            [
                outputs[k * BS : (k + 1) * BS][pa],
                outputs[B + k * BS : B + (k + 1) * BS][pa],
                outputs[2 * B + k * BS : 2 * B + (k + 1) * BS][pn],
            ],
            axis=0,
        )
        labsh = np.zeros((P, 128), np.float32)
        for t, v in enumerate((la_s, la_s, ln_s)):
            labsh[:, 16 * t : 16 * t + 16] = (
                (v.reshape(NB, P) - bases[:, None]).T.astype(np.float32)
            )
        maps.append(
            {
                "xout": _bf16(xo),
                "anc": _bf16(anchor[sl][pa].T),
                "pos": _bf16(positive[sl][pa].T),
                "neg": _bf16(negative[sl][pa].T),
                "exem": ex16,
                "idxa": _wrap_idx(la_s),
                "idxn": _wrap_idx(ln_pa),
                "labsh": labsh,
            }
        )
    return maps, tuple(int(b) for b in bases), W


def _combine(results):
    S = np.zeros(4, dtype=np.float64)
    for r in results:
        S += r["partials"][:, :4].astype(np.float64).sum(axis=0)
    loss_softmax = (S[0] - S[1]) / (3 * B)
    loss_center = S[2]
    loss_triplet = S[3]
    loss_total = loss_softmax + 0.01 * loss_center + LAMBDA * loss_triplet
    return (
        np.float32(loss_total),
        np.float32(loss_triplet),
        np.float32(loss_softmax),
        np.float32(loss_center),
    )


def kernel(anchor, positive, negative, outputs, labels_anchor, labels_neg, exemplars):
    global LAST_RESULTS
    maps, bases, W = _prep(
        anchor, positive, negative, outputs, labels_anchor, labels_neg, exemplars
    )
    nc = _build(bases, W)
    res = run_bass_kernel_spmd(nc, maps, core_ids=list(range(NCORES)))
    LAST_RESULTS = res
    return _combine(res.results)


# revision 23
# speedup vs baseline: 1.3994x; 1.0027x over previous
"""Trainium2 Bass kernel for nn_ExemplarSoftmaxLoss (data-parallel over 8 cores).

v4 strategy:
  - Host-side: rows of each core's shard are PERMUTED (all reductions are
    permutation-invariant): distance rows + xout thirds 0/1 sorted by
    labels_anchor, xout third 2 sorted by labels_neg.  Sorted rows make
    each 128-row block's labels fall in a narrow window, so the label-logit
    extraction only scans a static W-column window.  Window bases/width are
    computed from the data before compile (kernel builds lazily).
  - All bulk inputs are uploaded as bf16 (the 2e-2 rel-err budget makes
    mixed precision the right kernel design): halves HBM traffic to
    ~22.8 MB/core (~64 us of DMA) and enables the DVE 2x bf16 mode for
    the distance diffs.
  - The distance phase runs in a TRANSPOSED layout: anchor/pos/neg are
    uploaded as [D, BS] and exemplar rows are fetched with
    dma_gather(transpose=True), so diff tiles are [d-partition, row-free].
    Row sum-of-squares then runs on the otherwise-idle TensorEngine as
    diagonal matmuls df.T @ df (PSUM-accumulated over the 4 d-chunks);
    the diagonal is pulled out with a 128-wide is_equal STT.  This removes
    all 96 square ops (~60 us of Scalar+DVE) from the critical engines.
  - ScalarE runs the exp stream only; DVE does extraction + diffs + diag.
  - Host: float64 reduction of the 8x[128,4] partials -> 4 scalar losses.
"""

import os
import sys

import numpy as np
import ml_dtypes

for _p in ("/opt/trn_rl_repo",):
    if _p not in sys.path and os.path.isdir(_p):
        sys.path.insert(0, _p)

import concourse.bass as bass
import concourse.tile as tile
from concourse import bacc, mybir
from concourse._compat import with_exitstack
from concourse.bass_utils import run_bass_kernel_spmd

try:
    import antenv.axon_hooks  # noqa: F401
except ImportError:
    import types as _types

    _m = _types.ModuleType("antenv.axon_hooks")
    _m.get_axon_ntff_profile_hook = lambda: None
    _m.set_axon_ntff_profile_hook = lambda h: None
    sys.modules["antenv.axon_hooks"] = _m

# Problem constants (hardcoded per the harness contract).
B, D, C = 16384, 512, 1000
NCORES = 8
BS = B // NCORES  # 2048 batch rows per core
RS = 3 * BS  # 6144 softmax rows per core
P = 128
NB = BS // P  # 16 row-blocks in the distance phase
NR = RS // P  # 48 row-blocks in the softmax phase
NG = 4  # groups of 4 row-blocks in the distance phase
DC = D // P  # 4 d-chunks in the transposed layout
EPS = 1e-6
MARGIN2 = 0.2
LAMBDA = 1.0

# xout DMA tiles: n blocks each; 2-block head tiles give the exp stream an
# early start; the rest are 4-block (1MB bf16) tiles.
TILE_SHAPES = [2, 2, 4, 4, 4]
TILES = [
    (t, sum(TILE_SHAPES[:i]), TILE_SHAPES[i])
    for t in range(3)
    for i in range(len(TILE_SHAPES))
]
NXT = len(TILES)  # 15

f32 = mybir.dt.float32
bf16 = mybir.dt.bfloat16
i16 = mybir.dt.int16
Alu = mybir.AluOpType
Act = mybir.ActivationFunctionType
AX = mybir.AxisListType

LAST_RESULTS = None  # BassKernelResults of the most recent run (for test.py)


@with_exitstack
def _emit(ctx, tc, outs, ins, bases, W):
    nc = tc.nc
    xo = ins["xout"]  # [RS, C] bf16 (3 thirds, host-permuted)
    aa = ins["anc"]  # [D, BS] bf16 transposed (cols sorted by la)
    pp = ins["pos"]  # [D, BS] bf16
    ng = ins["neg"]  # [D, BS] bf16
    ex = ins["exem"]  # [C, D]  bf16 exemplar table
    ia = ins["idxa"]  # [128, 128] i16 wrapped gather idx (= sorted la)
    in_ = ins["idxn"]  # [128, 128] i16 wrapped gather idx (= ln[perm_a])
    lsh = ins["labsh"]  # [P, NR] f32 label - window_base per block
    pd = outs["partials"]  # [P, 4] f32

    sing = ctx.enter_context(tc.tile_pool(name="sing", bufs=1))
    xpool = ctx.enter_context(tc.tile_pool(name="xp", bufs=6))
    ejp = ctx.enter_context(tc.tile_pool(name="ejp", bufs=2, space="PSUM"))
    mmp = ctx.enter_context(tc.tile_pool(name="mmp", bufs=4, space="PSUM"))
    ljp = ctx.enter_context(tc.tile_pool(name="ljp", bufs=3))
    dgp = ctx.enter_context(tc.tile_pool(name="dgp", bufs=4))
    dfp = ctx.enter_context(tc.tile_pool(name="dfp", bufs=4))

    sums = sing.tile([P, NR], f32)  # per-row sum(exp(x))
    lbl = sing.tile([P, NR], f32)  # label logits per block
    d2a = sing.tile([P, NB * 3], f32)  # sq dists: dr1,dn1,dr2
    d2v = sing.tile([P, NB * 3], f32)  # sq dists: dn2,tp,tn
    iota_w = sing.tile([P, W], f32)
    pidx = sing.tile([P, 1], f32)  # value = partition index
    lsh_t = sing.tile([P, 128], f32)
    ia_t = sing.tile([128, 256], i16)
    in_t = sing.tile([128, 256], i16)
    # transposed distance operands: tile[p, c, r] = X[r, c*128+p]
    part = sing.tile([P, 128], f32)  # [:, :4] = loss partials
    at = sing.tile([P, DC, BS], bf16)
    pt = sing.tile([P, DC, BS], bf16)
    nt = sing.tile([P, DC, BS], bf16)
    # gather output must have contiguous free dims per call -> group-major
    exa = sing.tile([P, NG, DC, 512], bf16)
    exn = sing.tile([P, NG, DC, 512], bf16)

    # ALL DMAs ride the sync HWDGE queue: scalar-queue DMAs block the ACT
    # sequencer (delaying the exp stream), and the pool-dynamic queue is
    # starved while sync streams.  The small loads lead (padded to 512B
    # descriptors so they complete fast and don't stall the ring head).
    nc.sync.dma_start(out=lsh_t[:], in_=lsh[:])
    nc.sync.dma_start(out=ia_t[:], in_=ia[:])
    nc.sync.dma_start(out=in_t[:], in_=in_[:])
    nc.gpsimd.iota(
        iota_w[:],
        pattern=[[1, W]],
        base=0,
        channel_multiplier=0,
        allow_small_or_imprecise_dtypes=True,
    )
    nc.gpsimd.iota(
        pidx[:],
        pattern=[[1, 1]],
        base=0,
        channel_multiplier=1,
        allow_small_or_imprecise_dtypes=True,
    )
    for g in range(NG):
        for dst, idx in ((exa, ia_t), (exn, in_t)):
            nc.gpsimd.dma_gather(
                dst[:, g],
                ex[:],
                idx[:, 32 * g : 32 * g + 32],
                512,
                512,
                D,
                transpose=True,
            )

    nc.gpsimd.memset(part[:], 0.0)

    xo3 = xo.rearrange("(t r) c -> t r c", t=3)
    xt_tiles = {}

    def emit_xload(s):
        t, j0, nb = TILES[s]
        xt = xpool.tile([P, nb, C], bf16, tag="xt", name=f"xt{s}")
        nc.sync.dma_start(
            out=xt[:],
            in_=xo3[t, j0 * P : (j0 + nb) * P, :].rearrange(
                "(t p) c -> p t c", p=P
            ),
        )
        xt_tiles[s] = xt

    def emit_xcompute(s):
        xt = xt_tiles.pop(s)
        t, j0, nb = TILES[s]
        for b in range(nb):
            j = j0 + b  # block index within the third
            col = 16 * t + j
            ej = ejp.tile([P, C], f32, tag="ej")
            nc.scalar.activation(
                out=ej[:],
                in_=xt[:, b, :],
                func=Act.Exp,
                accum_out=sums[:, col : col + 1],
            )
            base = bases[j]
            lj = ljp.tile([P, W], f32, tag="lj")
            nc.vector.scalar_tensor_tensor(
                out=lj[:],
                in0=iota_w[:],
                scalar=lsh_t[:, col : col + 1],
                in1=xt[:, b, base : base + W],
                op0=Alu.is_equal,
                op1=Alu.mult,
                accum_out=lbl[:, col : col + 1],
            )

    def emit_apn_loads(g):
        r0, r1 = 512 * g, 512 * (g + 1)
        for dst, src in ((at, aa), (pt, pp), (nt, ng)):
            nc.sync.dma_start(
                out=dst[:, :, r0:r1],
                in_=src[:, r0:r1].rearrange("(c p) r -> p c r", p=P),
            )

    def emit_diag(df, rcl, d2t, ci, g):
        # mm = df_chunk.T @ df_chunk accumulated over the 4 d-chunks;
        # diag(mm)[p] = sum_d df[d, blk*128+p]^2 = d^2 of row blk*128+p
        blk = 4 * g + rcl
        rsl = slice(128 * rcl, 128 * (rcl + 1))
        mm = mmp.tile([P, P], f32, tag="mm")
        for dc in range(DC):
            nc.tensor.matmul(
                out=mm[:],
                lhsT=df[:, dc, rsl],
                rhs=df[:, dc, rsl],
                start=(dc == 0),
                stop=(dc == DC - 1),
            )
        dg = dgp.tile([P, P], f32, tag="dg")
        nc.vector.scalar_tensor_tensor(
            out=dg[:],
            in0=iota_w[:, 0:P],
            scalar=pidx[:],
            in1=mm[:],
            op0=Alu.is_equal,
            op1=Alu.mult,
            accum_out=d2t[:, blk * 3 + ci : blk * 3 + ci + 1],
        )

    def emit_group(g):
        rsl = slice(512 * g, 512 * (g + 1))
        pairs = (
            (at, exa, d2a, 0),  # d_ref1
            (nt, exa, d2a, 1),  # d_neg1
            (at, exn, d2a, 2),  # d_ref2
            (nt, exn, d2v, 0),  # d_neg2
            (at, pt, d2v, 1),  # tp
            (at, nt, d2v, 2),  # tn
        )
        dfs = []
        # software-pipeline: diffs lead their diag extraction by one pair so
        # the DVE never waits on the PE matmuls.  (Pool-engine TT was tried
        # here and regressed badly: ~4.5us per op plus SBUF-port contention
        # that halved the DVE 2x diff rate.)
        for pi, (xs, ys, d2t, ci) in enumerate(pairs):
            in0 = xs[:, :, rsl]
            in1 = ys[:, g] if (ys is exa or ys is exn) else ys[:, :, rsl]
            df = dfp.tile([P, DC, 512], bf16, tag="df")
            nc.vector.tensor_tensor(
                out=df[:], in0=in0, in1=in1, op=Alu.subtract
            )
            dfs.append((df, d2t, ci))
            if pi >= 1:
                df0, d2t0, ci0 = dfs[pi - 1]
                for rcl in range(4):
                    emit_diag(df0, rcl, d2t0, ci0, g)
        df0, d2t0, ci0 = dfs[-1]
        for rcl in range(4):
            emit_diag(df0, rcl, d2t0, ci0, g)

    # ---- main schedule ----
    emit_xload(0)
    emit_xload(1)
    emit_apn_loads(0)
    emit_xload(2)
    emit_apn_loads(1)

    for s in range(NXT):
        if s + 3 < NXT:
            emit_xload(s + 3)
        if s == 2:
            emit_apn_loads(2)
        if s == 4:
            emit_apn_loads(3)
        emit_xcompute(s)
        if s in (4, 6, 8, 10):
            emit_group((s - 4) // 2)

    # ---- tail ----
    dda = sing.tile([P, NB * 3], f32)
    ddv = sing.tile([P, NB * 3], f32)
    nc.scalar.activation(out=dda[:], in_=d2a[:], func=Act.Sqrt)
    nc.scalar.activation(out=ddv[:], in_=d2v[:], func=Act.Sqrt)
    logs = sing.tile([P, NR], f32)
    nc.scalar.activation(out=logs[:], in_=sums[:], func=Act.Ln)
    nc.vector.reduce_sum(out=part[:, 0:1], in_=logs[:], axis=AX.X)
    nc.vector.reduce_sum(out=part[:, 1:2], in_=lbl[:], axis=AX.X)

    dA = dda[:].rearrange("p (b k) -> p b k", k=3)
    dV = ddv[:].rearrange("p (b k) -> p b k", k=3)

    x1 = sing.tile([P, NB], f32)
    m1 = sing.tile([P, NB], f32)
    c1 = sing.tile([P, NB], f32)
    x2 = sing.tile([P, NB], f32)
    c2 = sing.tile([P, NB], f32)
    x3 = sing.tile([P, NB], f32)
    t3 = sing.tile([P, NB], f32)
    ca = sing.tile([P, 1], f32)
    cb = sing.tile([P, 1], f32)

    # c1 = (dr1 - dn1 > 0) ? (dr1 - dn1 + MARGIN2) : 0
    nc.vector.tensor_tensor(out=x1[:], in0=dA[:, :, 0], in1=dA[:, :, 1], op=Alu.subtract)
    nc.vector.tensor_scalar(
        out=m1[:], in0=x1[:], scalar1=0.0, scalar2=None, op0=Alu.is_gt
    )
    nc.vector.scalar_tensor_tensor(
        out=c1[:], in0=x1[:], scalar=MARGIN2, in1=m1[:],
        op0=Alu.add, op1=Alu.mult, accum_out=ca[:],
    )
    # c2 = relu(dn2 - dr2)
    nc.vector.tensor_tensor(out=x2[:], in0=dV[:, :, 0], in1=dA[:, :, 2], op=Alu.subtract)
    nc.vector.tensor_scalar(
        out=c2[:], in0=x2[:], scalar1=0.0, scalar2=None,
        op0=Alu.max, op1=Alu.add, accum_out=cb[:],
    )
    # t = relu(tp - tn)
    nc.vector.tensor_tensor(out=x3[:], in0=dV[:, :, 1], in1=dV[:, :, 2], op=Alu.subtract)
    nc.vector.tensor_scalar(
        out=t3[:], in0=x3[:], scalar1=0.0, scalar2=None,
        op0=Alu.max, op1=Alu.add, accum_out=part[:, 3:4],
    )
    nc.vector.tensor_tensor(out=part[:, 2:3], in0=ca[:], in1=cb[:], op=Alu.add)
    nc.sync.dma_start(out=pd[:], in_=part[:])


_COMPILED = {}


def _build(bases, W):
    key = (tuple(bases), W)
    if key in _COMPILED:
        return _COMPILED[key]
    nc = bacc.Bacc(
        "TRN2",
        target_bir_lowering=False,
        debug=False,
        enable_asserts=False,
        num_devices=NCORES,
    )
    ins = {
        "xout": nc.dram_tensor("xout", [RS, C], bf16, kind="ExternalInput").ap(),
        "anc": nc.dram_tensor("anc", [D, BS], bf16, kind="ExternalInput").ap(),
        "pos": nc.dram_tensor("pos", [D, BS], bf16, kind="ExternalInput").ap(),
        "neg": nc.dram_tensor("neg", [D, BS], bf16, kind="ExternalInput").ap(),
        "exem": nc.dram_tensor("exem", [C, D], bf16, kind="ExternalInput").ap(),
        "idxa": nc.dram_tensor("idxa", [128, 256], i16, kind="ExternalInput").ap(),
        "idxn": nc.dram_tensor("idxn", [128, 256], i16, kind="ExternalInput").ap(),
        "labsh": nc.dram_tensor("labsh", [P, 128], f32, kind="ExternalInput").ap(),
    }
    outs = {
        "partials": nc.dram_tensor("partials", [P, 128], f32, kind="ExternalOutput").ap()
    }
    with tile.TileContext(nc) as tc:
        _emit(tc, outs, ins, bases, W)
    nc.compile()
    _COMPILED[key] = nc
    return nc


def _wrap_idx(v):
    # dma_gather index layout: idx i at [i % 16, i // 16], replicated to
    # each 16-partition group (one per Q7 core).  Padded to 512B/partition
    # rows so the upload DMA avoids the sub-512B RMW descriptor penalty.
    w = np.asarray(v, np.int16).reshape(128, 16).T  # [16, 128]
    out = np.zeros((128, 256), np.int16)
    out[:, :128] = np.tile(w, (8, 1))
    return out


def _bf16(a):
    return np.ascontiguousarray(np.asarray(a, np.float32).astype(ml_dtypes.bfloat16))


def _prep(anchor, positive, negative, outputs, labels_anchor, labels_neg, exemplars):
    anchor = np.asarray(anchor, np.float32)
    positive = np.asarray(positive, np.float32)
    negative = np.asarray(negative, np.float32)
    outputs = np.asarray(outputs, np.float32)
    ex16 = _bf16(exemplars)
    la_all = np.asarray(labels_anchor).astype(np.int64)
    ln_all = np.asarray(labels_neg).astype(np.int64)

    cores = []
    lo = np.full(NB, C, np.int64)
    hi = np.full(NB, -1, np.int64)
    for k in range(NCORES):
        sl = slice(k * BS, (k + 1) * BS)
        la, ln = la_all[sl], ln_all[sl]
        pa = np.argsort(la, kind="stable")
        pn = np.argsort(ln, kind="stable")
        la_s, ln_s = la[pa], ln[pn]
        for v in (la_s, ln_s):
            vb = v.reshape(NB, P)
            np.minimum(lo, vb.min(axis=1), out=lo)
            np.maximum(hi, vb.max(axis=1), out=hi)
        cores.append((k, sl, pa, pn, la_s, ln_s))

    span = int((hi - lo).max()) + 1
    W = max(128, -(-span // 32) * 32)
    assert W <= C, f"label window infeasible: span {span}"
    bases = np.minimum(np.minimum(lo, C - W), hi - W + 1)
    bases = np.maximum(bases, 0).astype(np.int64)
    assert ((bases <= lo) & (bases + W > hi)).all()

    maps = []
    for k, sl, pa, pn, la_s, ln_s in cores:
        ln_pa = ln_all[sl][pa]
        xo = np.concatenate(
            [
                outputs[k * BS : (k + 1) * BS][pa],
                outputs[B + k * BS : B + (k + 1) * BS][pa],
                outputs[2 * B + k * BS : 2 * B + (k + 1) * BS][pn],
            ],
            axis=0,
        )
        labsh = np.zeros((P, 128), np.float32)
        for t, v in enumerate((la_s, la_s, ln_s)):
            labsh[:, 16 * t : 16 * t + 16] = (
                (v.reshape(NB, P) - bases[:, None]).T.astype(np.float32)
            )
        maps.append(
            {
                "xout": _bf16(xo),
                "anc": _bf16(anchor[sl][pa].T),
                "pos": _bf16(positive[sl][pa].T),
                "neg": _bf16(negative[sl][pa].T),
                "exem": ex16,
                "idxa": _wrap_idx(la_s),
                "idxn": _wrap_idx(ln_pa),
                "labsh": labsh,
            }
        )
    return maps, tuple(int(b) for b in bases), W


def _combine(results):
    S = np.zeros(4, dtype=np.float64)
    for r in results:
        S += r["partials"][:, :4].astype(np.float64).sum(axis=0)
    loss_softmax = (S[0] - S[1]) / (3 * B)
    loss_center = S[2]
    loss_triplet = S[3]
    loss_total = loss_softmax + 0.01 * loss_center + LAMBDA * loss_triplet
    return (
        np.float32(loss_total),
        np.float32(loss_triplet),
        np.float32(loss_softmax),
        np.float32(loss_center),
    )


def kernel(anchor, positive, negative, outputs, labels_anchor, labels_neg, exemplars):
    global LAST_RESULTS
    maps, bases, W = _prep(
        anchor, positive, negative, outputs, labels_anchor, labels_neg, exemplars
    )
    nc = _build(bases, W)
    res = run_bass_kernel_spmd(nc, maps, core_ids=list(range(NCORES)))
    LAST_RESULTS = res
    return _combine(res.results)


# revision 25
# speedup vs baseline: 1.4239x; 1.0176x over previous
"""Trainium2 Bass kernel for nn_ExemplarSoftmaxLoss (data-parallel over 8 cores).

v4 strategy:
  - Host-side: rows of each core's shard are PERMUTED (all reductions are
    permutation-invariant): distance rows + xout thirds 0/1 sorted by
    labels_anchor, xout third 2 sorted by labels_neg.  Sorted rows make
    each 128-row block's labels fall in a narrow window, so the label-logit
    extraction only scans a static W-column window.  Window bases/width are
    computed from the data before compile (kernel builds lazily).
  - All bulk inputs are uploaded as bf16 (the 2e-2 rel-err budget makes
    mixed precision the right kernel design): halves HBM traffic to
    ~22.8 MB/core (~64 us of DMA) and enables the DVE 2x bf16 mode for
    the distance diffs.
  - The distance phase runs in a TRANSPOSED layout: anchor/pos/neg are
    uploaded as [D, BS] and exemplar rows are fetched with
    dma_gather(transpose=True), so diff tiles are [d-partition, row-free].
    Row sum-of-squares then runs on the otherwise-idle TensorEngine as
    diagonal matmuls df.T @ df (PSUM-accumulated over the 4 d-chunks);
    the diagonal is pulled out with a 128-wide is_equal STT.  This removes
    all 96 square ops (~60 us of Scalar+DVE) from the critical engines.
  - ScalarE runs the exp stream only; DVE does extraction + diffs + diag.
  - Host: float64 reduction of the 8x[128,4] partials -> 4 scalar losses.
"""

import os
import sys

import numpy as np
import ml_dtypes

for _p in ("/opt/trn_rl_repo",):
    if _p not in sys.path and os.path.isdir(_p):
        sys.path.insert(0, _p)

import concourse.bass as bass
import concourse.tile as tile
from concourse import bacc, mybir
from concourse._compat import with_exitstack
from concourse.bass_utils import run_bass_kernel_spmd

try:
    import antenv.axon_hooks  # noqa: F401
except ImportError:
    import types as _types

    _m = _types.ModuleType("antenv.axon_hooks")
    _m.get_axon_ntff_profile_hook = lambda: None
    _m.set_axon_ntff_profile_hook = lambda h: None
    sys.modules["antenv.axon_hooks"] = _m

# Problem constants (hardcoded per the harness contract).
B, D, C = 16384, 512, 1000
NCORES = 8
BS = B // NCORES  # 2048 batch rows per core
RS = 3 * BS  # 6144 softmax rows per core
P = 128
NB = BS // P  # 16 row-blocks in the distance phase
NR = RS // P  # 48 row-blocks in the softmax phase
NG = 4  # groups of 4 row-blocks in the distance phase
DC = D // P  # 4 d-chunks in the transposed layout
EPS = 1e-6
MARGIN2 = 0.2
LAMBDA = 1.0

# xout DMA tiles: n blocks each; 2-block head tiles give the exp stream an
# early start; the rest are 4-block (1MB bf16) tiles.
TILE_SHAPES = [2, 2, 4, 4, 4]
TILES = [
    (t, sum(TILE_SHAPES[:i]), TILE_SHAPES[i])
    for t in range(3)
    for i in range(len(TILE_SHAPES))
]
NXT = len(TILES)  # 15

f32 = mybir.dt.float32
bf16 = mybir.dt.bfloat16
i16 = mybir.dt.int16
Alu = mybir.AluOpType
Act = mybir.ActivationFunctionType
AX = mybir.AxisListType

LAST_RESULTS = None  # BassKernelResults of the most recent run (for test.py)


@with_exitstack
def _emit(ctx, tc, outs, ins, bases, W):
    nc = tc.nc
    xo = ins["xout"]  # [RS, C] bf16 (3 thirds, host-permuted)
    aa = ins["anc"]  # [D, BS] bf16 transposed (cols sorted by la)
    pp = ins["pos"]  # [D, BS] bf16
    ng = ins["neg"]  # [D, BS] bf16
    ex = ins["exem"]  # [C, D]  bf16 exemplar table
    ia = ins["idxa"]  # [128, 128] i16 wrapped gather idx (= sorted la)
    in_ = ins["idxn"]  # [128, 128] i16 wrapped gather idx (= ln[perm_a])
    lsh = ins["labsh"]  # [P, NR] f32 label - window_base per block
    pd = outs["partials"]  # [P, 4] f32

    sing = ctx.enter_context(tc.tile_pool(name="sing", bufs=1))
    xpool = ctx.enter_context(tc.tile_pool(name="xp", bufs=6))
    ejp = ctx.enter_context(tc.tile_pool(name="ejp", bufs=2, space="PSUM"))
    mmp = ctx.enter_context(tc.tile_pool(name="mmp", bufs=4, space="PSUM"))
    ljp = ctx.enter_context(tc.tile_pool(name="ljp", bufs=3))
    dgp = ctx.enter_context(tc.tile_pool(name="dgp", bufs=4))
    dfp = ctx.enter_context(tc.tile_pool(name="dfp", bufs=4))

    sums = sing.tile([P, NR], f32)  # per-row sum(exp(x))
    lbl = sing.tile([P, NR], f32)  # label logits per block
    d2a = sing.tile([P, NB * 3], f32)  # sq dists: dr1,dn1,dr2
    d2v = sing.tile([P, NB * 3], f32)  # sq dists: dn2,tp,tn
    iota_w = sing.tile([P, W], f32)
    pidx = sing.tile([P, 1], f32)  # value = partition index
    lsh_t = sing.tile([P, 128], f32)
    ia_t = sing.tile([128, 256], i16)
    in_t = sing.tile([128, 256], i16)
    # transposed distance operands: tile[p, c, r] = X[r, c*128+p]
    part = sing.tile([P, 128], f32)  # [:, :4] = loss partials
    at = sing.tile([P, DC, BS], bf16)
    pt = sing.tile([P, DC, BS], bf16)
    nt = sing.tile([P, DC, BS], bf16)
    # gather output must have contiguous free dims per call -> group-major
    exa = sing.tile([P, NG, DC, 512], bf16)
    exn = sing.tile([P, NG, DC, 512], bf16)

    # Small loads ride the sync queue (every other SDMA queue is starved
    # while sync streams -- strict priority).  The tile scheduler would
    # reorder the xout tiles ahead of them (its SWDGE cost model thinks
    # gathers are cheap, so the idx loads look non-urgent, delaying the
    # first gather to ~21us).  The dummy Scalar copies below pin them
    # first: the Scalar queue is in-order and the exp stream follows, so
    # the scheduler must complete these DMAs before any exp.
    # (Padded to 512B descriptors to dodge the sub-512B RMW penalty.)
    nc.sync.dma_start(out=lsh_t[:], in_=lsh[:])
    nc.sync.dma_start(out=ia_t[:], in_=ia[:])
    nc.sync.dma_start(out=in_t[:], in_=in_[:])
    pin = sing.tile([P, 32], f32)
    nc.scalar.copy(out=pin[:], in_=lsh_t[:, 0:32])
    nc.scalar.copy(out=pin[:], in_=ia_t[:].bitcast(f32)[:, 0:32])
    nc.scalar.copy(out=pin[:], in_=in_t[:].bitcast(f32)[:, 0:32])
    nc.gpsimd.iota(
        iota_w[:],
        pattern=[[1, W]],
        base=0,
        channel_multiplier=0,
        allow_small_or_imprecise_dtypes=True,
    )
    nc.gpsimd.iota(
        pidx[:],
        pattern=[[1, 1]],
        base=0,
        channel_multiplier=1,
        allow_small_or_imprecise_dtypes=True,
    )
    for g in range(NG):
        for dst, idx in ((exa, ia_t), (exn, in_t)):
            nc.gpsimd.dma_gather(
                dst[:, g],
                ex[:],
                idx[:, 32 * g : 32 * g + 32],
                512,
                512,
                D,
                transpose=True,
            )

    nc.gpsimd.memset(part[:], 0.0)

    xo3 = xo.rearrange("(t r) c -> t r c", t=3)
    xt_tiles = {}

    def emit_xload(s):
        t, j0, nb = TILES[s]
        xt = xpool.tile([P, nb, C], bf16, tag="xt", name=f"xt{s}")
        nc.sync.dma_start(
            out=xt[:],
            in_=xo3[t, j0 * P : (j0 + nb) * P, :].rearrange(
                "(t p) c -> p t c", p=P
            ),
        )
        xt_tiles[s] = xt

    def emit_xcompute(s):
        xt = xt_tiles.pop(s)
        t, j0, nb = TILES[s]
        for b in range(nb):
            j = j0 + b  # block index within the third
            col = 16 * t + j
            ej = ejp.tile([P, C], f32, tag="ej")
            nc.scalar.activation(
                out=ej[:],
                in_=xt[:, b, :],
                func=Act.Exp,
                accum_out=sums[:, col : col + 1],
            )
            base = bases[j]
            lj = ljp.tile([P, W], f32, tag="lj")
            nc.vector.scalar_tensor_tensor(
                out=lj[:],
                in0=iota_w[:],
                scalar=lsh_t[:, col : col + 1],
                in1=xt[:, b, base : base + W],
                op0=Alu.is_equal,
                op1=Alu.mult,
                accum_out=lbl[:, col : col + 1],
            )

    def emit_apn_loads(g):
        r0, r1 = 512 * g, 512 * (g + 1)
        for dst, src in ((at, aa), (pt, pp), (nt, ng)):
            nc.sync.dma_start(
                out=dst[:, :, r0:r1],
                in_=src[:, r0:r1].rearrange("(c p) r -> p c r", p=P),
            )

    def emit_diag(df, rcl, d2t, ci, g):
        # mm = df_chunk.T @ df_chunk accumulated over the 4 d-chunks;
        # diag(mm)[p] = sum_d df[d, blk*128+p]^2 = d^2 of row blk*128+p
        blk = 4 * g + rcl
        rsl = slice(128 * rcl, 128 * (rcl + 1))
        mm = mmp.tile([P, P], f32, tag="mm")
        for dc in range(DC):
            nc.tensor.matmul(
                out=mm[:],
                lhsT=df[:, dc, rsl],
                rhs=df[:, dc, rsl],
                start=(dc == 0),
                stop=(dc == DC - 1),
            )
        dg = dgp.tile([P, P], f32, tag="dg")
        nc.vector.scalar_tensor_tensor(
            out=dg[:],
            in0=iota_w[:, 0:P],
            scalar=pidx[:],
            in1=mm[:],
            op0=Alu.is_equal,
            op1=Alu.mult,
            accum_out=d2t[:, blk * 3 + ci : blk * 3 + ci + 1],
        )

    def emit_group(g):
        rsl = slice(512 * g, 512 * (g + 1))
        pairs = (
            (at, exa, d2a, 0),  # d_ref1
            (nt, exa, d2a, 1),  # d_neg1
            (at, exn, d2a, 2),  # d_ref2
            (nt, exn, d2v, 0),  # d_neg2
            (at, pt, d2v, 1),  # tp
            (at, nt, d2v, 2),  # tn
        )
        dfs = []
        # software-pipeline: diffs lead their diag extraction by one pair so
        # the DVE never waits on the PE matmuls.  (Pool-engine TT was tried
        # here and regressed badly: ~4.5us per op plus SBUF-port contention
        # that halved the DVE 2x diff rate.)
        for pi, (xs, ys, d2t, ci) in enumerate(pairs):
            in0 = xs[:, :, rsl]
            in1 = ys[:, g] if (ys is exa or ys is exn) else ys[:, :, rsl]
            df = dfp.tile([P, DC, 512], bf16, tag="df")
            nc.vector.tensor_tensor(
                out=df[:], in0=in0, in1=in1, op=Alu.subtract
            )
            dfs.append((df, d2t, ci))
            if pi >= 1:
                df0, d2t0, ci0 = dfs[pi - 1]
                for rcl in range(4):
                    emit_diag(df0, rcl, d2t0, ci0, g)
        df0, d2t0, ci0 = dfs[-1]
        for rcl in range(4):
            emit_diag(df0, rcl, d2t0, ci0, g)

    # ---- main schedule ----
    emit_xload(0)
    emit_xload(1)
    emit_apn_loads(0)
    emit_xload(2)
    emit_apn_loads(1)

    for s in range(NXT):
        if s + 3 < NXT:
            emit_xload(s + 3)
        if s == 2:
            emit_apn_loads(2)
        if s == 4:
            emit_apn_loads(3)
        emit_xcompute(s)
        if s in (4, 6, 8, 10):
            emit_group((s - 4) // 2)

    # ---- tail ----
    dda = sing.tile([P, NB * 3], f32)
    ddv = sing.tile([P, NB * 3], f32)
    nc.scalar.activation(out=dda[:], in_=d2a[:], func=Act.Sqrt)
    nc.scalar.activation(out=ddv[:], in_=d2v[:], func=Act.Sqrt)
    logs = sing.tile([P, NR], f32)
    nc.scalar.activation(out=logs[:], in_=sums[:], func=Act.Ln)
    nc.vector.reduce_sum(out=part[:, 0:1], in_=logs[:], axis=AX.X)
    nc.vector.reduce_sum(out=part[:, 1:2], in_=lbl[:], axis=AX.X)

    dA = dda[:].rearrange("p (b k) -> p b k", k=3)
    dV = ddv[:].rearrange("p (b k) -> p b k", k=3)

    x1 = sing.tile([P, NB], f32)
    m1 = sing.tile([P, NB], f32)
    c1 = sing.tile([P, NB], f32)
    x2 = sing.tile([P, NB], f32)
    c2 = sing.tile([P, NB], f32)
    x3 = sing.tile([P, NB], f32)
    t3 = sing.tile([P, NB], f32)
    ca = sing.tile([P, 1], f32)
    cb = sing.tile([P, 1], f32)

    # c1 = (dr1 - dn1 > 0) ? (dr1 - dn1 + MARGIN2) : 0
    nc.vector.tensor_tensor(out=x1[:], in0=dA[:, :, 0], in1=dA[:, :, 1], op=Alu.subtract)
    nc.vector.tensor_scalar(
        out=m1[:], in0=x1[:], scalar1=0.0, scalar2=None, op0=Alu.is_gt
    )
    nc.vector.scalar_tensor_tensor(
        out=c1[:], in0=x1[:], scalar=MARGIN2, in1=m1[:],
        op0=Alu.add, op1=Alu.mult, accum_out=ca[:],
    )
    # c2 = relu(dn2 - dr2)
    nc.vector.tensor_tensor(out=x2[:], in0=dV[:, :, 0], in1=dA[:, :, 2], op=Alu.subtract)
    nc.vector.tensor_scalar(
        out=c2[:], in0=x2[:], scalar1=0.0, scalar2=None,
        op0=Alu.max, op1=Alu.add, accum_out=cb[:],
    )
    # t = relu(tp - tn)
    nc.vector.tensor_tensor(out=x3[:], in0=dV[:, :, 1], in1=dV[:, :, 2], op=Alu.subtract)
    nc.vector.tensor_scalar(
        out=t3[:], in0=x3[:], scalar1=0.0, scalar2=None,
        op0=Alu.max, op1=Alu.add, accum_out=part[:, 3:4],
    )
    nc.vector.tensor_tensor(out=part[:, 2:3], in0=ca[:], in1=cb[:], op=Alu.add)
    nc.sync.dma_start(out=pd[:], in_=part[:])


_COMPILED = {}


def _build(bases, W):
    key = (tuple(bases), W)
    if key in _COMPILED:
        return _COMPILED[key]
    nc = bacc.Bacc(
        "TRN2",
        target_bir_lowering=False,
        debug=False,
        enable_asserts=False,
        num_devices=NCORES,
    )
    ins = {
        "xout": nc.dram_tensor("xout", [RS, C], bf16, kind="ExternalInput").ap(),
        "anc": nc.dram_tensor("anc", [D, BS], bf16, kind="ExternalInput").ap(),
        "pos": nc.dram_tensor("pos", [D, BS], bf16, kind="ExternalInput").ap(),
        "neg": nc.dram_tensor("neg", [D, BS], bf16, kind="ExternalInput").ap(),
        "exem": nc.dram_tensor("exem", [C, D], bf16, kind="ExternalInput").ap(),
        "idxa": nc.dram_tensor("idxa", [128, 256], i16, kind="ExternalInput").ap(),
        "idxn": nc.dram_tensor("idxn", [128, 256], i16, kind="ExternalInput").ap(),
        "labsh": nc.dram_tensor("labsh", [P, 128], f32, kind="ExternalInput").ap(),
    }
    outs = {
        "partials": nc.dram_tensor("partials", [P, 128], f32, kind="ExternalOutput").ap()
    }
    with tile.TileContext(nc) as tc:
        _emit(tc, outs, ins, bases, W)
    nc.compile()
    _COMPILED[key] = nc
    return nc


def _wrap_idx(v):
    # dma_gather index layout: idx i at [i % 16, i // 16], replicated to
    # each 16-partition group (one per Q7 core).  Padded to 512B/partition
    # rows so the upload DMA avoids the sub-512B RMW descriptor penalty.
    w = np.asarray(v, np.int16).reshape(128, 16).T  # [16, 128]
    out = np.zeros((128, 256), np.int16)
    out[:, :128] = np.tile(w, (8, 1))
    return out


def _bf16(a):
    return np.ascontiguousarray(np.asarray(a, np.float32).astype(ml_dtypes.bfloat16))


def _prep(anchor, positive, negative, outputs, labels_anchor, labels_neg, exemplars):
    anchor = np.asarray(anchor, np.float32)
    positive = np.asarray(positive, np.float32)
    negative = np.asarray(negative, np.float32)
    outputs = np.asarray(outputs, np.float32)
    ex16 = _bf16(exemplars)
    la_all = np.asarray(labels_anchor).astype(np.int64)
    ln_all = np.asarray(labels_neg).astype(np.int64)

    cores = []
    lo = np.full(NB, C, np.int64)
    hi = np.full(NB, -1, np.int64)
    for k in range(NCORES):
        sl = slice(k * BS, (k + 1) * BS)
        la, ln = la_all[sl], ln_all[sl]
        pa = np.argsort(la, kind="stable")
        pn = np.argsort(ln, kind="stable")
        la_s, ln_s = la[pa], ln[pn]
        for v in (la_s, ln_s):
            vb = v.reshape(NB, P)
            np.minimum(lo, vb.min(axis=1), out=lo)
            np.maximum(hi, vb.max(axis=1), out=hi)
        cores.append((k, sl, pa, pn, la_s, ln_s))

    span = int((hi - lo).max()) + 1
    W = max(128, -(-span // 32) * 32)
    assert W <= C, f"label window infeasible: span {span}"
    bases = np.minimum(np.minimum(lo, C - W), hi - W + 1)
    bases = np.maximum(bases, 0).astype(np.int64)
    assert ((bases <= lo) & (bases + W > hi)).all()

    maps = []
    for k, sl, pa, pn, la_s, ln_s in cores:
        ln_pa = ln_all[sl][pa]
        xo = np.concatenate(
            [
                outputs[k * BS : (k + 1) * BS][pa],
                outputs[B + k * BS : B + (k + 1) * BS][pa],
                outputs[2 * B + k * BS : 2 * B + (k + 1) * BS][pn],
            ],
            axis=0,
        )
        labsh = np.zeros((P, 128), np.float32)
        for t, v in enumerate((la_s, la_s, ln_s)):
            labsh[:, 16 * t : 16 * t + 16] = (
                (v.reshape(NB, P) - bases[:, None]).T.astype(np.float32)
            )
        maps.append(
            {
                "xout": _bf16(xo),
                "anc": _bf16(anchor[sl][pa].T),
                "pos": _bf16(positive[sl][pa].T),
                "neg": _bf16(negative[sl][pa].T),
                "exem": ex16,
                "idxa": _wrap_idx(la_s),
                "idxn": _wrap_idx(ln_pa),
                "labsh": labsh,
            }
        )
    return maps, tuple(int(b) for b in bases), W


def _combine(results):
    S = np.zeros(4, dtype=np.float64)
    for r in results:
        S += r["partials"][:, :4].astype(np.float64).sum(axis=0)
    loss_softmax = (S[0] - S[1]) / (3 * B)
    loss_center = S[2]
    loss_triplet = S[3]
    loss_total = loss_softmax + 0.01 * loss_center + LAMBDA * loss_triplet
    return (
        np.float32(loss_total),
        np.float32(loss_triplet),
        np.float32(loss_softmax),
        np.float32(loss_center),
    )


def kernel(anchor, positive, negative, outputs, labels_anchor, labels_neg, exemplars):
    global LAST_RESULTS
    maps, bases, W = _prep(
        anchor, positive, negative, outputs, labels_anchor, labels_neg, exemplars
    )
    nc = _build(bases, W)
    res = run_bass_kernel_spmd(nc, maps, core_ids=list(range(NCORES)))
    LAST_RESULTS = res
    return _combine(res.results)
